# revision 5
# baseline (speedup 1.0000x reference)
"""VQ codebook lookup (AudioQuantizer) for Trainium2, 8-core data-parallel.

Full problem: x [65536, 512] f32, codebook [1024, 512] f32,
embedding_table [1024, 512] f32 -> out [65536, 512] f32 where
out[n] = embedding_table[argmin_k ||x[n] - codebook[k]||^2].

Sharding: x/out split along N across 8 cores; codebook + embedding table
replicated (2 MB each). No collectives.

Math: argmin_k ||x-c_k||^2 == argmax_k (x . c_k - |c_k|^2/2); the |x|^2 row
term is constant per row and dropped. Scores are computed on the PE with an
fp16 hi/lo split (x ~= hi+lo carries ~22 mantissa bits):
    cross = hi.c_hi + hi.c_lo + lo.c_hi      (3 fp16 matmuls / chunk,
residual ~2^-22, vs 4 cyc/row for native fp32 — measured 0/65536 argmin
flips against the fp32 jax reference, output bit-exact).

Per-core kernel, per group of 4 x-tiles (512 rows):
  one 1 MB DMA x group -> SBUF [128p, 4t, 512d]
  per tile: PE transposes (4x 128x128 via identity) -> xT in PSUM
            ACT round-copy -> xt_hi fp16; DVE subtract -> xt_lo fp16
            PE: scores[128, 1024] = K_c=2 ones-fold(-|c|^2/2 hi/lo)
                + hi/lo matmuls vs pre-transposed fp16 codebook
            ACT copy scores PSUM->SBUF; DVE Max8 + MaxIndex -> uint32 idx
            GPSIMD indirect-DMA gather emb[idx] -> group SBUF tile
  one 1 MB DMA group store -> out
"""

from contextlib import ExitStack

import numpy as np

import concourse.bass as bass
import concourse.mybir as mybir
import concourse.tile as tile
from concourse import bacc
from concourse.bass_utils import run_bass_kernel_spmd
from concourse.masks import make_identity

P = 128
D = 512
K = 1024
N_TOTAL = 65536
N_CORES = 8
N_PER_CORE = N_TOTAL // N_CORES
DC = D // P  # 4 contraction chunks of 128
KH = K // 512  # 2 score halves of 512
XG = 4  # x tiles per DMA group


def _build_kernel() -> bass.Bass:
    nc = bacc.Bacc("TRN2", target_bir_lowering=False, num_devices=N_CORES)
    f32 = mybir.dt.float32
    f16 = mybir.dt.float16

    x = nc.dram_tensor("x", [N_PER_CORE, D], f32, kind="ExternalInput")
    cb = nc.dram_tensor("codebook", [K, D], f32, kind="ExternalInput")
    emb = nc.dram_tensor("embedding_table", [K, D], f32, kind="ExternalInput")
    out = nc.dram_tensor("out", [N_PER_CORE, D], f32, kind="ExternalOutput")

    n_tiles = N_PER_CORE // P

    with ExitStack() as ctx:
        tc = ctx.enter_context(tile.TileContext(nc))
        const = ctx.enter_context(tc.tile_pool(name="const", bufs=1))
        xin_pool = ctx.enter_context(tc.tile_pool(name="xin", bufs=3))
        xt_pool = ctx.enter_context(tc.tile_pool(name="xt", bufs=3))
        sc_pool = ctx.enter_context(tc.tile_pool(name="scores", bufs=3))
        gather_pool = ctx.enter_context(tc.tile_pool(name="gather", bufs=3))
        small_pool = ctx.enter_context(tc.tile_pool(name="small", bufs=3))

        # setup-only PSUM pool in its own scope so its banks free afterwards;
        # bufs=4 lets the 32 codebook transposes pipeline instead of stalling
        # on a single PSUM slot.
        setup_ctx = ExitStack()
        setup_ps = setup_ctx.enter_context(
            tc.tile_pool(name="setup_ps", bufs=4, space="PSUM")
        )

        # ---- one-time setup ----
        identity = const.tile([P, P], f32, tag="identity")
        make_identity(nc, identity[:])

        ones_col = const.tile([P, 1], f32, tag="ones_col")
        nc.vector.memset(ones_col[:], 1.0)

        # codebook natural layout: [p, ko, d] with k = ko*P + p.
        # Load per-ko so the first transposes start after ~256 KB, not 2 MB.
        cb_sb = const.tile([P, K // P, D], f32, tag="cb_sb")
        cb_r = cb.ap().rearrange("(ko p) d -> p ko d", p=P)
        for ko in range(K // P):
            nc.sync.dma_start(cb_sb[:, ko : ko + 1, :], cb_r[:, ko : ko + 1, :])

        # transposed codebook split hi/lo fp16: cbT[p_d, dc, k] = cb[k, dc*P+p_d]
        cbT_hi = const.tile([P, DC, K], f16, tag="cbT_hi")
        cbT_lo = const.tile([P, DC, K], f16, tag="cbT_lo")
        cbT_f32 = const.tile([P, DC, K], f32, tag="cbT_f32")
        cbT_sq = const.tile([P, DC, K], f32, tag="cbT_sq")
        for ko in range(K // P):
            for dc in range(DC):
                t_ps = setup_ps.tile([P, P], f32, tag="t_ps")
                nc.tensor.transpose(
                    t_ps[:], cb_sb[:, ko, dc * P : (dc + 1) * P], identity[:]
                )
                ksl = slice(ko * P, (ko + 1) * P)
                nc.scalar.copy(cbT_hi[:, dc, ksl], t_ps[:])
                nc.vector.tensor_sub(cbT_lo[:, dc, ksl], t_ps[:], cbT_hi[:, dc, ksl])
                # exact c~ = hi + lo so |c|^2 below matches what the matmuls see
                nc.vector.tensor_add(
                    cbT_f32[:, dc, ksl], cbT_hi[:, dc, ksl], cbT_lo[:, dc, ksl]
                )
                nc.vector.tensor_mul(
                    cbT_sq[:, dc, ksl], cbT_f32[:, dc, ksl], cbT_f32[:, dc, ksl]
                )

        # -|c|^2/2 in row layout via fp32 ones-matmul over squared cbT
        ncsq_f32 = const.tile([1, K], f32, tag="ncsq_f32")
        for h in range(KH):
            csq_ps = setup_ps.tile([1, 512], f32, tag="csq_ps")
            for dc in range(DC):
                nc.tensor.matmul(
                    csq_ps[:],
                    lhsT=ones_col[:],
                    rhs=cbT_sq[:, dc, h * 512 : (h + 1) * 512],
                    start=(dc == 0),
                    stop=(dc == DC - 1),
                )
            nc.scalar.mul(ncsq_f32[0:1, h * 512 : (h + 1) * 512], csq_ps[:], -0.5)

        # fold operands: ones [2, P] fp16 and -|c|^2/2 split hi/lo [2, K] fp16.
        # Engines can't write at partition offset 1, so build the rows in
        # [1, K] tiles and assemble with two small DMAs.
        ones_f = const.tile([2, P], f16, tag="ones_f")
        nc.vector.memset(ones_f[:], 1.0)
        ncsq_hi16 = const.tile([1, K], f16, tag="ncsq_hi16")
        nc.scalar.copy(ncsq_hi16[:], ncsq_f32[0:1, :])
        ncsq_lo16 = const.tile([1, K], f16, tag="ncsq_lo16")
        nc.vector.tensor_sub(ncsq_lo16[:], ncsq_f32[0:1, :], ncsq_hi16[:])
        ncsq = const.tile([2, K], f16, tag="ncsq")
        nc.sync.dma_start(ncsq[0:1, :], ncsq_hi16[:])
        nc.sync.dma_start(ncsq[1:2, :], ncsq_lo16[:])

        setup_ctx.close()
        xt_ps = ctx.enter_context(tc.tile_pool(name="xt_ps", bufs=2, space="PSUM"))
        sc_ps = ctx.enter_context(tc.tile_pool(name="sc_ps", bufs=4, space="PSUM"))

        # ---- main loop: groups of XG tiles; batched 1 MB x-load / out-store ----
        for g in range(n_tiles // XG):
            xg_sb = xin_pool.tile([P, XG, D], f32, tag="x_sb")
            nc.sync.dma_start(
                xg_sb[:],
                x.ap()[g * XG * P : (g + 1) * XG * P, :].rearrange(
                    "(t p) d -> p t d", p=P
                ),
            )
            rows_g = gather_pool.tile([P, XG, D], f32, tag="rows")
            for t in range(XG):
                xt_psum = xt_ps.tile([P, D], f32, tag="xt_psum")
                for dc in range(DC):
                    nc.tensor.transpose(
                        xt_psum[:, dc * P : (dc + 1) * P],
                        xg_sb[:, t, dc * P : (dc + 1) * P],
                        identity[:],
                    )
                xt3 = xt_psum[:].rearrange("p (dc n) -> p dc n", dc=DC)
                xt_hi = xt_pool.tile([P, DC, P], f16, tag="xt_hi")
                nc.scalar.copy(xt_hi[:], xt3)
                xt_lo = xt_pool.tile([P, DC, P], f16, tag="xt_lo")
                nc.vector.tensor_sub(xt_lo[:], xt3, xt_hi[:])

                score_sb = sc_pool.tile([P, K], f32, tag="score_sb")
                for h in range(KH):
                    ksl = slice(h * 512, (h + 1) * 512)
                    s_ps = sc_ps.tile([P, 512], f32, tag="s_ps")
                    nc.tensor.matmul(
                        s_ps[:], lhsT=ones_f[:], rhs=ncsq[:, ksl],
                        start=True, stop=False,
                    )
                    # hi-based matmuls first (need only the fast ACT copy);
                    # lo-based last so the DVE subtract has slack
                    for dc in range(DC):
                        nc.tensor.matmul(
                            s_ps[:], lhsT=xt_hi[:, dc, :], rhs=cbT_hi[:, dc, ksl],
                            start=False, stop=False,
                        )
                        nc.tensor.matmul(
                            s_ps[:], lhsT=xt_hi[:, dc, :], rhs=cbT_lo[:, dc, ksl],
                            start=False, stop=False,
                        )
                    for dc in range(DC):
                        nc.tensor.matmul(
                            s_ps[:], lhsT=xt_lo[:, dc, :], rhs=cbT_hi[:, dc, ksl],
                            start=False, stop=(dc == DC - 1),
                        )
                    nc.scalar.copy(score_sb[:, ksl], s_ps[:])

                max8 = small_pool.tile([P, 8], f32, tag="max8")
                nc.vector.max(max8[:], score_sb[:])
                idx = small_pool.tile([P, 8], mybir.dt.uint32, tag="idx")
                nc.vector.max_index(idx[:], max8[:], score_sb[:])

                nc.gpsimd.indirect_dma_start(
                    out=rows_g[:, t, :],
                    out_offset=None,
                    in_=emb.ap()[:],
                    in_offset=bass.IndirectOffsetOnAxis(ap=idx[:, 0:1], axis=0),
                )
            nc.sync.dma_start(
                out.ap()[g * XG * P : (g + 1) * XG * P, :].rearrange(
                    "(t p) d -> p t d", p=P
                ),
                rows_g[:],
            )

    nc.compile()
    return nc


_NC_CACHE: list = []


def _run_once(inputs: dict) -> np.ndarray:
    x = np.ascontiguousarray(np.asarray(inputs["x"], dtype=np.float32))
    cb = np.ascontiguousarray(np.asarray(inputs["codebook"], dtype=np.float32))
    emb = np.ascontiguousarray(
        np.asarray(inputs["embedding_table"], dtype=np.float32)
    )
    assert x.shape == (N_TOTAL, D) and cb.shape == (K, D) and emb.shape == (K, D)

    if not _NC_CACHE:
        _NC_CACHE.append(_build_kernel())
    nc = _NC_CACHE[0]

    in_maps = [
        {
            "x": x[c * N_PER_CORE : (c + 1) * N_PER_CORE],
            "codebook": cb,
            "embedding_table": emb,
        }
        for c in range(N_CORES)
    ]
    res = run_bass_kernel_spmd(nc, in_maps, core_ids=list(range(N_CORES)))
    return np.concatenate([r["out"] for r in res.results], axis=0)


def _run_in_subprocess(inputs: dict) -> np.ndarray:
    """Fresh-process retry: a transient NRT_EXEC_UNIT_UNRECOVERABLE fault
    poisons this process's PJRT client, but a fresh process recovers."""
    import os
    import subprocess
    import sys
    import tempfile

    kdir = os.path.dirname(os.path.abspath(__file__))
    with tempfile.TemporaryDirectory() as td:
        np.savez(os.path.join(td, "in.npz"), **inputs)
        script = (
            "import sys, numpy as np\n"
            f"sys.path.insert(0, {kdir!r})\n"
            "import os\n"
            "os.environ['VQ_KERNEL_NO_SUBPROC'] = '1'\n"
            "import kernel as k\n"
            f"d = dict(np.load(os.path.join({td!r}, 'in.npz')))\n"
            "out = k.kernel(**d)\n"
            f"np.save(os.path.join({td!r}, 'out.npy'), out)\n"
        )
        subprocess.run(
            [sys.executable, "-c", script], check=True, timeout=1800
        )
        return np.load(os.path.join(td, "out.npy"))


def kernel(**inputs: np.ndarray) -> np.ndarray:
    import os
    import time as _time

    try:
        return _run_once(inputs)
    except Exception:
        if os.environ.get("VQ_KERNEL_NO_SUBPROC"):
            # already the retry subprocess: one more in-process attempt
            _time.sleep(5)
            return _run_once(inputs)
        # The failed PJRT client can't be revived in-process; a fresh
        # process clears transient device wedges reliably.
        _time.sleep(5)
        return _run_in_subprocess(inputs)


# revision 6
# speedup vs baseline: 1.0013x; 1.0013x over previous
"""VQ codebook lookup (AudioQuantizer) for Trainium2, 8-core data-parallel.

Full problem: x [65536, 512] f32, codebook [1024, 512] f32,
embedding_table [1024, 512] f32 -> out [65536, 512] f32 where
out[n] = embedding_table[argmin_k ||x[n] - codebook[k]||^2].

Sharding: x/out split along N across 8 cores; codebook + embedding table
replicated (2 MB each). No collectives.

Math: argmin_k ||x-c_k||^2 == argmax_k (x . c_k - |c_k|^2/2); the |x|^2 row
term is constant per row and dropped. Scores are computed on the PE with an
fp16 hi/lo split (x ~= hi+lo carries ~22 mantissa bits):
    cross = hi.c_hi + hi.c_lo + lo.c_hi      (3 fp16 matmuls / chunk,
residual ~2^-22, vs 4 cyc/row for native fp32 — measured 0/65536 argmin
flips against the fp32 jax reference, output bit-exact).

Per-core kernel, per group of 4 x-tiles (512 rows):
  one 1 MB DMA x group -> SBUF [128p, 4t, 512d]
  per tile: PE transposes (4x 128x128 via identity) -> xT in PSUM
            ACT round-copy -> xt_hi fp16; DVE subtract -> xt_lo fp16
            PE: scores[128, 1024] = K_c=2 ones-fold(-|c|^2/2 hi/lo)
                + hi/lo matmuls vs pre-transposed fp16 codebook
            ACT copy scores PSUM->SBUF; DVE Max8 + MaxIndex -> uint32 idx
            GPSIMD indirect-DMA gather emb[idx] -> group SBUF tile
  one 1 MB DMA group store -> out
"""

from contextlib import ExitStack

import numpy as np

import concourse.bass as bass
import concourse.mybir as mybir
import concourse.tile as tile
from concourse import bacc
from concourse.bass_utils import run_bass_kernel_spmd
from concourse.masks import make_identity

P = 128
D = 512
K = 1024
N_TOTAL = 65536
N_CORES = 8
N_PER_CORE = N_TOTAL // N_CORES
DC = D // P  # 4 contraction chunks of 128
KH = K // 512  # 2 score halves of 512
XG = 4  # x tiles per DMA group


def _build_kernel() -> bass.Bass:
    nc = bacc.Bacc("TRN2", target_bir_lowering=False, num_devices=N_CORES)
    f32 = mybir.dt.float32
    f16 = mybir.dt.float16

    x = nc.dram_tensor("x", [N_PER_CORE, D], f32, kind="ExternalInput")
    cb = nc.dram_tensor("codebook", [K, D], f32, kind="ExternalInput")
    emb = nc.dram_tensor("embedding_table", [K, D], f32, kind="ExternalInput")
    out = nc.dram_tensor("out", [N_PER_CORE, D], f32, kind="ExternalOutput")

    n_tiles = N_PER_CORE // P

    with ExitStack() as ctx:
        tc = ctx.enter_context(tile.TileContext(nc))
        const = ctx.enter_context(tc.tile_pool(name="const", bufs=1))
        xin_pool = ctx.enter_context(tc.tile_pool(name="xin", bufs=3))
        xt_pool = ctx.enter_context(tc.tile_pool(name="xt", bufs=3))
        sc_pool = ctx.enter_context(tc.tile_pool(name="scores", bufs=3))
        gather_pool = ctx.enter_context(tc.tile_pool(name="gather", bufs=3))
        small_pool = ctx.enter_context(tc.tile_pool(name="small", bufs=3))

        # setup-only PSUM pool in its own scope so its banks free afterwards;
        # bufs=4 lets the 32 codebook transposes pipeline instead of stalling
        # on a single PSUM slot.
        setup_ctx = ExitStack()
        setup_ps = setup_ctx.enter_context(
            tc.tile_pool(name="setup_ps", bufs=4, space="PSUM")
        )

        # ---- one-time setup ----
        identity = const.tile([P, P], f32, tag="identity")
        make_identity(nc, identity[:])

        ones_col = const.tile([P, 1], f32, tag="ones_col")
        nc.vector.memset(ones_col[:], 1.0)

        # codebook natural layout: [p, ko, d] with k = ko*P + p.
        # Load per-ko so the first transposes start after ~256 KB, not 2 MB.
        cb_sb = const.tile([P, K // P, D], f32, tag="cb_sb")
        cb_r = cb.ap().rearrange("(ko p) d -> p ko d", p=P)
        for ko in range(K // P):
            nc.sync.dma_start(cb_sb[:, ko : ko + 1, :], cb_r[:, ko : ko + 1, :])

        # transposed codebook split hi/lo fp16: cbT[p_d, dc, k] = cb[k, dc*P+p_d]
        cbT_hi = const.tile([P, DC, K], f16, tag="cbT_hi")
        cbT_lo = const.tile([P, DC, K], f16, tag="cbT_lo")
        cbT_f32 = const.tile([P, DC, K], f32, tag="cbT_f32")
        cbT_sq = const.tile([P, DC, K], f32, tag="cbT_sq")
        for ko in range(K // P):
            for dc in range(DC):
                t_ps = setup_ps.tile([P, P], f32, tag="t_ps")
                nc.tensor.transpose(
                    t_ps[:], cb_sb[:, ko, dc * P : (dc + 1) * P], identity[:]
                )
                ksl = slice(ko * P, (ko + 1) * P)
                nc.scalar.copy(cbT_hi[:, dc, ksl], t_ps[:])
                nc.vector.tensor_sub(cbT_lo[:, dc, ksl], t_ps[:], cbT_hi[:, dc, ksl])
                # exact c~ = hi + lo so |c|^2 below matches what the matmuls see
                nc.vector.tensor_add(
                    cbT_f32[:, dc, ksl], cbT_hi[:, dc, ksl], cbT_lo[:, dc, ksl]
                )
                nc.vector.tensor_mul(
                    cbT_sq[:, dc, ksl], cbT_f32[:, dc, ksl], cbT_f32[:, dc, ksl]
                )

        # -|c|^2/2 in row layout via fp32 ones-matmul over squared cbT
        ncsq_f32 = const.tile([1, K], f32, tag="ncsq_f32")
        for h in range(KH):
            csq_ps = setup_ps.tile([1, 512], f32, tag="csq_ps")
            for dc in range(DC):
                nc.tensor.matmul(
                    csq_ps[:],
                    lhsT=ones_col[:],
                    rhs=cbT_sq[:, dc, h * 512 : (h + 1) * 512],
                    start=(dc == 0),
                    stop=(dc == DC - 1),
                )
            nc.scalar.mul(ncsq_f32[0:1, h * 512 : (h + 1) * 512], csq_ps[:], -0.5)

        # fold operands: ones [2, P] fp16 and -|c|^2/2 split hi/lo [2, K] fp16.
        # Engines can't write at partition offset 1, so build the rows in
        # [1, K] tiles and assemble with two small DMAs.
        ones_f = const.tile([2, P], f16, tag="ones_f")
        nc.vector.memset(ones_f[:], 1.0)
        ncsq_hi16 = const.tile([1, K], f16, tag="ncsq_hi16")
        nc.scalar.copy(ncsq_hi16[:], ncsq_f32[0:1, :])
        ncsq_lo16 = const.tile([1, K], f16, tag="ncsq_lo16")
        nc.vector.tensor_sub(ncsq_lo16[:], ncsq_f32[0:1, :], ncsq_hi16[:])
        ncsq = const.tile([2, K], f16, tag="ncsq")
        nc.sync.dma_start(ncsq[0:1, :], ncsq_hi16[:])
        nc.sync.dma_start(ncsq[1:2, :], ncsq_lo16[:])

        setup_ctx.close()
        xt_ps = ctx.enter_context(tc.tile_pool(name="xt_ps", bufs=3, space="PSUM"))
        sc_ps = ctx.enter_context(tc.tile_pool(name="sc_ps", bufs=5, space="PSUM"))

        # ---- main loop: groups of XG tiles; batched 1 MB x-load / out-store ----
        for g in range(n_tiles // XG):
            xg_sb = xin_pool.tile([P, XG, D], f32, tag="x_sb")
            nc.sync.dma_start(
                xg_sb[:],
                x.ap()[g * XG * P : (g + 1) * XG * P, :].rearrange(
                    "(t p) d -> p t d", p=P
                ),
            )
            rows_g = gather_pool.tile([P, XG, D], f32, tag="rows")
            for t in range(XG):
                xt_psum = xt_ps.tile([P, D], f32, tag="xt_psum")
                for dc in range(DC):
                    nc.tensor.transpose(
                        xt_psum[:, dc * P : (dc + 1) * P],
                        xg_sb[:, t, dc * P : (dc + 1) * P],
                        identity[:],
                    )
                xt3 = xt_psum[:].rearrange("p (dc n) -> p dc n", dc=DC)
                xt_hi = xt_pool.tile([P, DC, P], f16, tag="xt_hi")
                nc.scalar.copy(xt_hi[:], xt3)
                xt_lo = xt_pool.tile([P, DC, P], f16, tag="xt_lo")
                nc.vector.tensor_sub(xt_lo[:], xt3, xt_hi[:])

                score_sb = sc_pool.tile([P, K], f32, tag="score_sb")
                for h in range(KH):
                    ksl = slice(h * 512, (h + 1) * 512)
                    s_ps = sc_ps.tile([P, 512], f32, tag="s_ps")
                    nc.tensor.matmul(
                        s_ps[:], lhsT=ones_f[:], rhs=ncsq[:, ksl],
                        start=True, stop=False,
                    )
                    # hi-based matmuls first (need only the fast ACT copy);
                    # lo-based last so the DVE subtract has slack
                    for dc in range(DC):
                        nc.tensor.matmul(
                            s_ps[:], lhsT=xt_hi[:, dc, :], rhs=cbT_hi[:, dc, ksl],
                            start=False, stop=False,
                        )
                        nc.tensor.matmul(
                            s_ps[:], lhsT=xt_hi[:, dc, :], rhs=cbT_lo[:, dc, ksl],
                            start=False, stop=False,
                        )
                    for dc in range(DC):
                        nc.tensor.matmul(
                            s_ps[:], lhsT=xt_lo[:, dc, :], rhs=cbT_hi[:, dc, ksl],
                            start=False, stop=(dc == DC - 1),
                        )
                    nc.scalar.copy(score_sb[:, ksl], s_ps[:])

                max8 = small_pool.tile([P, 8], f32, tag="max8")
                nc.vector.max(max8[:], score_sb[:])
                idx = small_pool.tile([P, 8], mybir.dt.uint32, tag="idx")
                nc.vector.max_index(idx[:], max8[:], score_sb[:])

                nc.gpsimd.indirect_dma_start(
                    out=rows_g[:, t, :],
                    out_offset=None,
                    in_=emb.ap()[:],
                    in_offset=bass.IndirectOffsetOnAxis(ap=idx[:, 0:1], axis=0),
                )
            nc.sync.dma_start(
                out.ap()[g * XG * P : (g + 1) * XG * P, :].rearrange(
                    "(t p) d -> p t d", p=P
                ),
                rows_g[:],
            )

    nc.compile()
    return nc


_NC_CACHE: list = []


def _run_once(inputs: dict) -> np.ndarray:
    x = np.ascontiguousarray(np.asarray(inputs["x"], dtype=np.float32))
    cb = np.ascontiguousarray(np.asarray(inputs["codebook"], dtype=np.float32))
    emb = np.ascontiguousarray(
        np.asarray(inputs["embedding_table"], dtype=np.float32)
    )
    assert x.shape == (N_TOTAL, D) and cb.shape == (K, D) and emb.shape == (K, D)

    if not _NC_CACHE:
        _NC_CACHE.append(_build_kernel())
    nc = _NC_CACHE[0]

    in_maps = [
        {
            "x": x[c * N_PER_CORE : (c + 1) * N_PER_CORE],
            "codebook": cb,
            "embedding_table": emb,
        }
        for c in range(N_CORES)
    ]
    res = run_bass_kernel_spmd(nc, in_maps, core_ids=list(range(N_CORES)))
    return np.concatenate([r["out"] for r in res.results], axis=0)


def _run_in_subprocess(inputs: dict) -> np.ndarray:
    """Fresh-process retry: a transient NRT_EXEC_UNIT_UNRECOVERABLE fault
    poisons this process's PJRT client, but a fresh process recovers."""
    import os
    import subprocess
    import sys
    import tempfile

    kdir = os.path.dirname(os.path.abspath(__file__))
    with tempfile.TemporaryDirectory() as td:
        np.savez(os.path.join(td, "in.npz"), **inputs)
        script = (
            "import sys, numpy as np\n"
            f"sys.path.insert(0, {kdir!r})\n"
            "import os\n"
            "os.environ['VQ_KERNEL_NO_SUBPROC'] = '1'\n"
            "import kernel as k\n"
            f"d = dict(np.load(os.path.join({td!r}, 'in.npz')))\n"
            "out = k.kernel(**d)\n"
            f"np.save(os.path.join({td!r}, 'out.npy'), out)\n"
        )
        subprocess.run(
            [sys.executable, "-c", script], check=True, timeout=1800
        )
        return np.load(os.path.join(td, "out.npy"))


def kernel(**inputs: np.ndarray) -> np.ndarray:
    import os
    import time as _time

    try:
        return _run_once(inputs)
    except Exception:
        if os.environ.get("VQ_KERNEL_NO_SUBPROC"):
            # already the retry subprocess: one more in-process attempt
            _time.sleep(5)
            return _run_once(inputs)
        # The failed PJRT client can't be revived in-process; a fresh
        # process clears transient device wedges reliably.
        _time.sleep(5)
        return _run_in_subprocess(inputs)


# revision 7
# speedup vs baseline: 1.0071x; 1.0058x over previous
"""VQ codebook lookup (AudioQuantizer) for Trainium2, 8-core data-parallel.

Full problem: x [65536, 512] f32, codebook [1024, 512] f32,
embedding_table [1024, 512] f32 -> out [65536, 512] f32 where
out[n] = embedding_table[argmin_k ||x[n] - codebook[k]||^2].

Sharding: x/out split along N across 8 cores; codebook + embedding table
replicated (2 MB each). No collectives.

Math: argmin_k ||x-c_k||^2 == argmax_k (x . c_k - |c_k|^2/2); the |x|^2 row
term is constant per row and dropped. Scores are computed on the PE with an
fp16 hi/lo split (x ~= hi+lo carries ~22 mantissa bits):
    cross = hi.c_hi + hi.c_lo + lo.c_hi      (3 fp16 matmuls / chunk,
residual ~2^-22, vs 4 cyc/row for native fp32 — measured 0/65536 argmin
flips against the fp32 jax reference, output bit-exact).

Per-core kernel, per group of 4 x-tiles (512 rows):
  one 1 MB DMA x group -> SBUF [128p, 4t, 512d]
  per tile: PE transposes (4x 128x128 via identity) -> xT in PSUM
            ACT round-copy -> xt_hi fp16; DVE subtract -> xt_lo fp16
            PE: scores[128, 1024] = K_c=2 ones-fold(-|c|^2/2 hi/lo)
                + hi/lo matmuls vs pre-transposed fp16 codebook
            ACT copy scores PSUM->SBUF; DVE Max8 + MaxIndex -> uint32 idx
            GPSIMD indirect-DMA gather emb[idx] -> group SBUF tile
  one 1 MB DMA group store -> out
"""

from contextlib import ExitStack

import numpy as np

import concourse.bass as bass
import concourse.mybir as mybir
import concourse.tile as tile
from concourse import bacc
from concourse.bass_utils import run_bass_kernel_spmd
from concourse.masks import make_identity

P = 128
D = 512
K = 1024
N_TOTAL = 65536
N_CORES = 8
N_PER_CORE = N_TOTAL // N_CORES
DC = D // P  # 4 contraction chunks of 128
KH = K // 512  # 2 score halves of 512
XG = 4  # x tiles per DMA group


def _build_kernel() -> bass.Bass:
    nc = bacc.Bacc("TRN2", target_bir_lowering=False, num_devices=N_CORES)
    f32 = mybir.dt.float32
    f16 = mybir.dt.float16

    x = nc.dram_tensor("x", [N_PER_CORE, D], f32, kind="ExternalInput")
    cb = nc.dram_tensor("codebook", [K, D], f32, kind="ExternalInput")
    emb = nc.dram_tensor("embedding_table", [K, D], f32, kind="ExternalInput")
    out = nc.dram_tensor("out", [N_PER_CORE, D], f32, kind="ExternalOutput")

    n_tiles = N_PER_CORE // P

    with ExitStack() as ctx:
        tc = ctx.enter_context(tile.TileContext(nc))
        const = ctx.enter_context(tc.tile_pool(name="const", bufs=1))
        xin_pool = ctx.enter_context(tc.tile_pool(name="xin", bufs=3))
        xt_pool = ctx.enter_context(tc.tile_pool(name="xt", bufs=3))
        sc_pool = ctx.enter_context(tc.tile_pool(name="scores", bufs=3))
        gather_pool = ctx.enter_context(tc.tile_pool(name="gather", bufs=3))
        small_pool = ctx.enter_context(tc.tile_pool(name="small", bufs=3))

        # setup-only PSUM pool in its own scope so its banks free afterwards;
        # bufs=4 lets the 32 codebook transposes pipeline instead of stalling
        # on a single PSUM slot.
        setup_ctx = ExitStack()
        setup_ps = setup_ctx.enter_context(
            tc.tile_pool(name="setup_ps", bufs=4, space="PSUM")
        )

        # ---- one-time setup ----
        identity = const.tile([P, P], f32, tag="identity")
        make_identity(nc, identity[:])

        ones_col = const.tile([P, 1], f32, tag="ones_col")
        nc.vector.memset(ones_col[:], 1.0)

        # codebook natural layout: [p, ko, d] with k = ko*P + p.
        # Load per-ko so the first transposes start after ~256 KB, not 2 MB.
        cb_sb = const.tile([P, K // P, D], f32, tag="cb_sb")
        cb_r = cb.ap().rearrange("(ko p) d -> p ko d", p=P)
        for ko in range(K // P):
            nc.sync.dma_start(cb_sb[:, ko : ko + 1, :], cb_r[:, ko : ko + 1, :])

        # transposed codebook split hi/lo fp16: cbT[p_d, dc, k] = cb[k, dc*P+p_d]
        cbT_hi = const.tile([P, DC, K], f16, tag="cbT_hi")
        cbT_lo = const.tile([P, DC, K], f16, tag="cbT_lo")
        cbT_f32 = const.tile([P, DC, K], f32, tag="cbT_f32")
        cbT_sq = const.tile([P, DC, K], f32, tag="cbT_sq")
        for ko in range(K // P):
            for dc in range(DC):
                t_ps = setup_ps.tile([P, P], f32, tag="t_ps")
                nc.tensor.transpose(
                    t_ps[:], cb_sb[:, ko, dc * P : (dc + 1) * P], identity[:]
                )
                ksl = slice(ko * P, (ko + 1) * P)
                nc.scalar.copy(cbT_hi[:, dc, ksl], t_ps[:])
                nc.vector.tensor_sub(cbT_lo[:, dc, ksl], t_ps[:], cbT_hi[:, dc, ksl])
                # exact c~ = hi + lo so |c|^2 below matches what the matmuls see
                nc.vector.tensor_add(
                    cbT_f32[:, dc, ksl], cbT_hi[:, dc, ksl], cbT_lo[:, dc, ksl]
                )
                nc.vector.tensor_mul(
                    cbT_sq[:, dc, ksl], cbT_f32[:, dc, ksl], cbT_f32[:, dc, ksl]
                )

        # -|c|^2/2 in row layout via fp32 ones-matmul over squared cbT
        ncsq_f32 = const.tile([1, K], f32, tag="ncsq_f32")
        for h in range(KH):
            csq_ps = setup_ps.tile([1, 512], f32, tag="csq_ps")
            for dc in range(DC):
                nc.tensor.matmul(
                    csq_ps[:],
                    lhsT=ones_col[:],
                    rhs=cbT_sq[:, dc, h * 512 : (h + 1) * 512],
                    start=(dc == 0),
                    stop=(dc == DC - 1),
                )
            nc.scalar.mul(ncsq_f32[0:1, h * 512 : (h + 1) * 512], csq_ps[:], -0.5)

        # fold operands: ones [2, P] fp16 and -|c|^2/2 split hi/lo [2, K] fp16.
        # Engines can't write at partition offset 1, so build the rows in
        # [1, K] tiles and assemble with two small DMAs.
        ones_f = const.tile([2, P], f16, tag="ones_f")
        nc.vector.memset(ones_f[:], 1.0)
        ncsq_hi16 = const.tile([1, K], f16, tag="ncsq_hi16")
        nc.scalar.copy(ncsq_hi16[:], ncsq_f32[0:1, :])
        ncsq_lo16 = const.tile([1, K], f16, tag="ncsq_lo16")
        nc.vector.tensor_sub(ncsq_lo16[:], ncsq_f32[0:1, :], ncsq_hi16[:])
        ncsq = const.tile([2, K], f16, tag="ncsq")
        nc.sync.dma_start(ncsq[0:1, :], ncsq_hi16[:])
        nc.sync.dma_start(ncsq[1:2, :], ncsq_lo16[:])

        setup_ctx.close()
        xt_ps = ctx.enter_context(tc.tile_pool(name="xt_ps", bufs=3, space="PSUM"))
        sc_ps = ctx.enter_context(tc.tile_pool(name="sc_ps", bufs=5, space="PSUM"))

        # ---- main loop: groups of XG tiles; batched 1 MB x-load / out-store.
        # The last group stores per-tile instead, shortening the kernel tail:
        # earlier tiles flush while later ones still compute, and the final
        # store is 256 KB, not 1 MB. ----
        n_groups = n_tiles // XG
        for g in range(n_groups):
            xg_sb = xin_pool.tile([P, XG, D], f32, tag="x_sb")
            nc.sync.dma_start(
                xg_sb[:],
                x.ap()[g * XG * P : (g + 1) * XG * P, :].rearrange(
                    "(t p) d -> p t d", p=P
                ),
            )
            last_group = g == n_groups - 1
            if not last_group:
                rows_g = gather_pool.tile([P, XG, D], f32, tag="rows")
            for t in range(XG):
                xt_psum = xt_ps.tile([P, D], f32, tag="xt_psum")
                for dc in range(DC):
                    nc.tensor.transpose(
                        xt_psum[:, dc * P : (dc + 1) * P],
                        xg_sb[:, t, dc * P : (dc + 1) * P],
                        identity[:],
                    )
                xt3 = xt_psum[:].rearrange("p (dc n) -> p dc n", dc=DC)
                xt_hi = xt_pool.tile([P, DC, P], f16, tag="xt_hi")
                nc.scalar.copy(xt_hi[:], xt3)
                xt_lo = xt_pool.tile([P, DC, P], f16, tag="xt_lo")
                nc.vector.tensor_sub(xt_lo[:], xt3, xt_hi[:])

                score_sb = sc_pool.tile([P, K], f32, tag="score_sb")
                for h in range(KH):
                    ksl = slice(h * 512, (h + 1) * 512)
                    s_ps = sc_ps.tile([P, 512], f32, tag="s_ps")
                    nc.tensor.matmul(
                        s_ps[:], lhsT=ones_f[:], rhs=ncsq[:, ksl],
                        start=True, stop=False,
                    )
                    # hi-based matmuls first (need only the fast ACT copy);
                    # lo-based last so the DVE subtract has slack
                    for dc in range(DC):
                        nc.tensor.matmul(
                            s_ps[:], lhsT=xt_hi[:, dc, :], rhs=cbT_hi[:, dc, ksl],
                            start=False, stop=False,
                        )
                        nc.tensor.matmul(
                            s_ps[:], lhsT=xt_hi[:, dc, :], rhs=cbT_lo[:, dc, ksl],
                            start=False, stop=False,
                        )
                    for dc in range(DC):
                        nc.tensor.matmul(
                            s_ps[:], lhsT=xt_lo[:, dc, :], rhs=cbT_hi[:, dc, ksl],
                            start=False, stop=(dc == DC - 1),
                        )
                    nc.scalar.copy(score_sb[:, ksl], s_ps[:])

                max8 = small_pool.tile([P, 8], f32, tag="max8")
                nc.vector.max(max8[:], score_sb[:])
                idx = small_pool.tile([P, 8], mybir.dt.uint32, tag="idx")
                nc.vector.max_index(idx[:], max8[:], score_sb[:])

                if last_group:
                    rows_t = gather_pool.tile([P, D], f32, tag="rows_t")
                    nc.gpsimd.indirect_dma_start(
                        out=rows_t[:],
                        out_offset=None,
                        in_=emb.ap()[:],
                        in_offset=bass.IndirectOffsetOnAxis(ap=idx[:, 0:1], axis=0),
                    )
                    i = g * XG + t
                    nc.sync.dma_start(out.ap()[i * P : (i + 1) * P, :], rows_t[:])
                else:
                    nc.gpsimd.indirect_dma_start(
                        out=rows_g[:, t, :],
                        out_offset=None,
                        in_=emb.ap()[:],
                        in_offset=bass.IndirectOffsetOnAxis(ap=idx[:, 0:1], axis=0),
                    )
            if not last_group:
                nc.sync.dma_start(
                    out.ap()[g * XG * P : (g + 1) * XG * P, :].rearrange(
                        "(t p) d -> p t d", p=P
                    ),
                    rows_g[:],
                )

    nc.compile()
    return nc


_NC_CACHE: list = []


def _run_once(inputs: dict) -> np.ndarray:
    x = np.ascontiguousarray(np.asarray(inputs["x"], dtype=np.float32))
    cb = np.ascontiguousarray(np.asarray(inputs["codebook"], dtype=np.float32))
    emb = np.ascontiguousarray(
        np.asarray(inputs["embedding_table"], dtype=np.float32)
    )
    assert x.shape == (N_TOTAL, D) and cb.shape == (K, D) and emb.shape == (K, D)

    if not _NC_CACHE:
        _NC_CACHE.append(_build_kernel())
    nc = _NC_CACHE[0]

    in_maps = [
        {
            "x": x[c * N_PER_CORE : (c + 1) * N_PER_CORE],
            "codebook": cb,
            "embedding_table": emb,
        }
        for c in range(N_CORES)
    ]
    res = run_bass_kernel_spmd(nc, in_maps, core_ids=list(range(N_CORES)))
    return np.concatenate([r["out"] for r in res.results], axis=0)


def _run_in_subprocess(inputs: dict) -> np.ndarray:
    """Fresh-process retry: a transient NRT_EXEC_UNIT_UNRECOVERABLE fault
    poisons this process's PJRT client, but a fresh process recovers."""
    import os
    import subprocess
    import sys
    import tempfile

    kdir = os.path.dirname(os.path.abspath(__file__))
    with tempfile.TemporaryDirectory() as td:
        np.savez(os.path.join(td, "in.npz"), **inputs)
        script = (
            "import sys, numpy as np\n"
            f"sys.path.insert(0, {kdir!r})\n"
            "import os\n"
            "os.environ['VQ_KERNEL_NO_SUBPROC'] = '1'\n"
            "import kernel as k\n"
            f"d = dict(np.load(os.path.join({td!r}, 'in.npz')))\n"
            "out = k.kernel(**d)\n"
            f"np.save(os.path.join({td!r}, 'out.npy'), out)\n"
        )
        subprocess.run(
            [sys.executable, "-c", script], check=True, timeout=1800
        )
        return np.load(os.path.join(td, "out.npy"))


def kernel(**inputs: np.ndarray) -> np.ndarray:
    import os
    import time as _time

    try:
        return _run_once(inputs)
    except Exception:
        if os.environ.get("VQ_KERNEL_NO_SUBPROC"):
            # already the retry subprocess: one more in-process attempt
            _time.sleep(5)
            return _run_once(inputs)
        # The failed PJRT client can't be revived in-process; a fresh
        # process clears transient device wedges reliably.
        _time.sleep(5)
        return _run_in_subprocess(inputs)
